# revision 5
# baseline (speedup 1.0000x reference)
"""Trainium2 Bass kernel for nn_AttentionBlock (causal attention, column softmax).

Computation (reference):
    Q/K/V = X @ W + b  per batch b of X[4, 4096, 512]
    logits[t,s] = <q_t, k_s>, causal mask (s>t -> -inf),
    probs = softmax over t (per column s) / sqrt(512)
    out = X + probs @ V

Sharding: 8 cores = (batch b in 0..3) x (half h in 0..1). Within a batch the
32 key-blocks (128 rows each) are split between the two halves so that both
halves get two blocks of every "extent class" c (blocks 4c..4c+3 share the
query window [512c, 4096)), giving an identical SPMD program on every core
with balanced causal work. Masks are data, not program structure.

Per core: Q^T = Wq^T X^T (full T), K^T/V for its 2048 key rows, then per
key-block: logits^T = K_blk^T.T Q^T over the block's query window, exp via
ACT (row-sums via accum_out), diagonal-block mask via DVE multiply,
reciprocal row-sums folded into V rows, then AV matmuls accumulate the
partial output per 128-query block in PSUM. Host adds the two partials and
the residual. All matmuls bf16 with fp32 accumulation.
"""
import sys
if "/opt/trn_rl_repo" not in sys.path:
    sys.path.insert(0, "/opt/trn_rl_repo")

import numpy as np
import ml_dtypes

import concourse.bass as bass  # noqa: F401  (bass must import before tile)
import concourse.tile as tile
from concourse import bacc, mybir
from concourse.bass_utils import run_bass_kernel_spmd

bf16 = ml_dtypes.bfloat16
AFT = mybir.ActivationFunctionType
ALU = mybir.AluOpType

B, T, D = 4, 4096, 512      # K = V = D = 512
P = 128                     # partitions
NSLOT = 16                  # key blocks per core
CH = 512                    # chunk width (free dim per matmul)
INV_SQRT_K = float(1.0 / np.sqrt(np.float32(D)))

# slot i -> class c = i//2; extent (in 512-chunks) E = 8 - c; window start 512c
_EXT = [(8 - (i // 2)) * CH for i in range(NSLOT)]
_POFF = np.concatenate([[0], np.cumsum(_EXT)]).astype(int)  # offsets into pall
PTOT = int(_POFF[-1])  # 36864


def _build_program(reps=1):
    nc = bacc.Bacc("TRN2", target_bir_lowering=False, debug=False, num_devices=8)
    dbf, df32 = mybir.dt.bfloat16, mybir.dt.float32

    XT = nc.dram_tensor("XT", [D, T], dbf, kind="ExternalInput").ap()
    XST = nc.dram_tensor("XST", [D, 2048], dbf, kind="ExternalInput").ap()
    WQ = nc.dram_tensor("WQ", [D, D], dbf, kind="ExternalInput").ap()
    WK = nc.dram_tensor("WK", [D, D], dbf, kind="ExternalInput").ap()
    WV = nc.dram_tensor("WV", [D, D], dbf, kind="ExternalInput").ap()
    BQ = nc.dram_tensor("BQ", [P, 4], df32, kind="ExternalInput").ap()
    BK = nc.dram_tensor("BK", [P, 4], df32, kind="ExternalInput").ap()
    BV = nc.dram_tensor("BV", [P, D], dbf, kind="ExternalInput").ap()
    MASK = nc.dram_tensor("MASK", [2, P, CH], dbf, kind="ExternalInput").ap()
    OUT = nc.dram_tensor("OUT", [T, D], df32, kind="ExternalOutput").ap()

    with tile.TileContext(nc) as tc:
        with tc.tile_pool(name="persist", bufs=1) as pp, \
             tc.tile_pool(name="small", bufs=2) as sp, \
             tc.tile_pool(name="lpsum", bufs=4, space="PSUM") as lp, \
             tc.tile_pool(name="cpsum", bufs=2, space="PSUM") as cp:

            qt = pp.tile([P, 4 * T], dbf, tag="qt")        # Q^T: [kq | t]
            kt = pp.tile([P, 4 * 2048], dbf, tag="kt")     # K^T: [kq | s_local]
            vsc = pp.tile([P, NSLOT * CH], dbf, tag="vsc")  # V rows (later scaled)
            pall = pp.tile([P, PTOT], dbf, tag="pall")     # exp(logits^T) all slots
            bq = pp.tile([P, 4], df32, tag="bq")
            bk = pp.tile([P, 4], df32, tag="bk")
            bv = pp.tile([P, D], dbf, tag="bv")
            mask = pp.tile([P, 2 * CH], dbf, tag="mask")

            def one_rep(rep):
                with tc.tile_pool(name=f"aph{rep}", bufs=1) as ap_, \
                     tc.tile_pool(name=f"apsum{rep}", bufs=2, space="PSUM") as aps:
                    xt = ap_.tile([P, 4 * T], dbf, tag="xt")       # X^T: [d | t]
                    xst = ap_.tile([P, 4 * 2048], dbf, tag="xst")  # X_sel^T: [d | s]
                    wq = ap_.tile([P, 4 * D], dbf, tag="wq")       # [d | kq]
                    wk = ap_.tile([P, 4 * D], dbf, tag="wk")
                    wv = ap_.tile([P, 4 * D], dbf, tag="wv")

                    for db in range(4):
                        nc.sync.dma_start(xt[:, T * db:T * (db + 1)], XT[P * db:P * (db + 1), :])
                    for db in range(4):
                        nc.sync.dma_start(wq[:, D * db:D * (db + 1)], WQ[P * db:P * (db + 1), :])
                        nc.sync.dma_start(wk[:, D * db:D * (db + 1)], WK[P * db:P * (db + 1), :])
                        nc.sync.dma_start(wv[:, D * db:D * (db + 1)], WV[P * db:P * (db + 1), :])
                    for db in range(4):
                        nc.sync.dma_start(xst[:, 2048 * db:2048 * (db + 1)], XST[P * db:P * (db + 1), :])
                    nc.sync.dma_start(bq[:], BQ[:])
                    nc.sync.dma_start(bk[:], BK[:])
                    nc.sync.dma_start(bv[:], BV[:])
                    for r in range(2):
                        nc.sync.dma_start(mask[:, CH * r:CH * (r + 1)], MASK[r])

                    # Q^T[kq, t] = sum_d Wq[d, kq] X^T[d, t]  (+ bq per partition)
                    for kb in range(4):
                        for g in range(8):
                            ps = aps.tile([P, CH], df32, tag="aps")
                            for db in range(4):
                                nc.tensor.matmul(
                                    ps[:],
                                    wq[:, D * db + P * kb: D * db + P * kb + P],
                                    xt[:, T * db + CH * g: T * db + CH * (g + 1)],
                                    start=(db == 0), stop=(db == 3),
                                )
                            nc.scalar.activation(
                                qt[:, T * kb + CH * g: T * kb + CH * (g + 1)], ps[:],
                                AFT.Identity, bias=bq[:, kb:kb + 1],
                            )
                    # K^T[kq, s] likewise over the 2048 selected rows
                    for kb in range(4):
                        for sc in range(4):
                            ps = aps.tile([P, CH], df32, tag="aps")
                            for db in range(4):
                                nc.tensor.matmul(
                                    ps[:],
                                    wk[:, D * db + P * kb: D * db + P * kb + P],
                                    xst[:, 2048 * db + CH * sc: 2048 * db + CH * (sc + 1)],
                                    start=(db == 0), stop=(db == 3),
                                )
                            nc.scalar.activation(
                                kt[:, 2048 * kb + CH * sc: 2048 * kb + CH * (sc + 1)], ps[:],
                                AFT.Identity, bias=bk[:, kb:kb + 1],
                            )
                    # V[s, v] = sum_d X_sel[s, d] Wv[d, v]  (+ bv broadcast)
                    for i in range(NSLOT):
                        ps = aps.tile([P, CH], df32, tag="aps")
                        for db in range(4):
                            nc.tensor.matmul(
                                ps[:],
                                xst[:, 2048 * db + P * i: 2048 * db + P * i + P],
                                wv[:, D * db: D * (db + 1)],
                                start=(db == 0), stop=(db == 3),
                            )
                        nc.vector.tensor_add(vsc[:, CH * i:CH * (i + 1)], ps[:], bv[:])

                # Phase B: per slot logits -> exp -> row sums -> fold 1/denom into V
                for i in range(NSLOT):
                    c, r = i // 2, i % 2
                    E = 8 - c
                    off = int(_POFF[i])
                    sums = sp.tile([P, 8], df32, tag="sums")
                    for e in range(E):
                        g = c + e
                        ps = lp.tile([P, CH], df32, tag="lg")
                        for kb in range(4):
                            nc.tensor.matmul(
                                ps[:],
                                kt[:, 2048 * kb + P * i: 2048 * kb + P * i + P],
                                qt[:, T * kb + CH * g: T * kb + CH * (g + 1)],
                                start=(kb == 0), stop=(kb == 3),
                            )
                        if e == 0:
                            ptmp = sp.tile([P, CH], mybir.dt.bfloat16, tag="ptmp")
                            nc.scalar.activation(ptmp[:], ps[:], AFT.Exp)
                            nc.vector.tensor_mul(
                                pall[:, off:off + CH], ptmp[:], mask[:, CH * r:CH * (r + 1)]
                            )
                            nc.vector.tensor_reduce(
                                sums[:, 0:1], pall[:, off:off + CH],
                                axis=mybir.AxisListType.X, op=ALU.add,
                            )
                        else:
                            nc.scalar.activation(
                                pall[:, off + CH * e: off + CH * (e + 1)], ps[:],
                                AFT.Exp, accum_out=sums[:, e:e + 1],
                            )
                    den = sp.tile([P, 1], df32, tag="den")
                    nc.vector.tensor_reduce(den[:], sums[:, 0:E], axis=mybir.AxisListType.X, op=ALU.add)
                    r2 = sp.tile([P, 1], df32, tag="r2")
                    nc.vector.reciprocal(r2[:], den[:])
                    nc.vector.tensor_scalar(
                        out=vsc[:, CH * i:CH * (i + 1)], in0=vsc[:, CH * i:CH * (i + 1)],
                        scalar1=r2[:], scalar2=INV_SQRT_K,
                        op0=ALU.mult, op1=ALU.mult,
                    )

                # Phase C: out[t-block] = sum_slots P_slot^T V_slot  (PSUM accumulate)
                for tau in range(32):
                    cmax = tau // 4
                    n = 2 * (cmax + 1)
                    ps = cp.tile([P, CH], df32, tag="avp")
                    for i in range(n):
                        c = i // 2
                        tloc = tau - 4 * c
                        po = int(_POFF[i]) + P * tloc
                        nc.tensor.matmul(
                            ps[:], pall[:, po:po + P], vsc[:, CH * i:CH * (i + 1)],
                            start=(i == 0), stop=(i == n - 1),
                        )
                    st = sp.tile([P, CH], df32, tag="st")
                    nc.vector.tensor_copy(st[:], ps[:])
                    nc.sync.dma_start(OUT[P * tau:P * (tau + 1), :], st[:])

            for rep in range(reps):
                one_rep(rep)

    nc.compile()
    return nc


_PROGRAM = None


def _get_program():
    global _PROGRAM
    if _PROGRAM is None:
        _PROGRAM = _build_program()
    return _PROGRAM


def _core_inputs(X, Wq_b, Wk_b, Wv_b, BQ_h, BK_h, BV_b, masks, b, h):
    """Per-core input map for core (b, h)."""
    Xb = X[b]
    XTb = np.ascontiguousarray(Xb.T).astype(bf16)
    sel = Xb.reshape(8, 2, 256, D)[:, h].reshape(2048, D)
    XSTb = np.ascontiguousarray(sel.T).astype(bf16)
    return {
        "XT": XTb, "XST": XSTb,
        "WQ": Wq_b, "WK": Wk_b, "WV": Wv_b,
        "BQ": BQ_h, "BK": BK_h, "BV": BV_b,
        "MASK": masks[h],
    }


def _prep_shared(Wk, bk, Wq, bq, Wv, bv):
    Wq_b = np.ascontiguousarray(np.asarray(Wq)).astype(bf16)
    Wk_b = np.ascontiguousarray(np.asarray(Wk)).astype(bf16)
    Wv_b = np.ascontiguousarray(np.asarray(Wv)).astype(bf16)
    BQ_h = np.ascontiguousarray(np.asarray(bq, np.float32).reshape(4, P).T)
    BK_h = np.ascontiguousarray(np.asarray(bk, np.float32).reshape(4, P).T)
    BV_b = np.tile(np.asarray(bv).astype(bf16)[None, :], (P, 1))
    masks = np.zeros((2, 2, P, CH), dtype=bf16)  # [h][r]
    s_loc = np.arange(P)[:, None]
    t_loc = np.arange(CH)[None, :]
    for h in range(2):
        for r in range(2):
            q = 2 * h + r
            masks[h, r] = (t_loc >= P * q + s_loc).astype(bf16)
    return Wq_b, Wk_b, Wv_b, BQ_h, BK_h, BV_b, masks


def kernel(minibatch, Wk, bk, Wq, bq, Wv, bv):
    X = np.asarray(minibatch, dtype=np.float32)
    nc = _get_program()
    Wq_b, Wk_b, Wv_b, BQ_h, BK_h, BV_b, masks = _prep_shared(Wk, bk, Wq, bq, Wv, bv)
    in_maps = [
        _core_inputs(X, Wq_b, Wk_b, Wv_b, BQ_h, BK_h, BV_b, masks, b, h)
        for b in range(B) for h in range(2)
    ]
    res = run_bass_kernel_spmd(nc, in_maps, list(range(2 * B)))
    out = X.copy()
    for b in range(B):
        out[b] += res.results[2 * b]["OUT"]
        out[b] += res.results[2 * b + 1]["OUT"]
    return out
